# revision 4
# baseline (speedup 1.0000x reference)
"""Llama GQA attention (B=2, S=2048, E=2048, H=32, KVH=8, D=64) on 8 TRN2 cores.

Sharding: data-parallel over batch x tensor-parallel over heads. Core c
handles batch c//4 and q-heads 8*(c%4)..8*(c%4)+7 (kv heads 2*(c%4)+{0,1}).
o_proj is row-parallel; the 4 partial outputs per batch are summed on host.

kernel(**inputs) takes the full unsharded inputs and returns
(attn_output [B,S,E] f32, attn_weights [B,H,S,S] f32) like the reference.
"""
import sys
sys.path.insert(0, "/opt/trn_rl_repo")

import numpy as np
import ml_dtypes

import concourse.bass as bass
import concourse.bacc as bacc
import concourse.tile as tile
from concourse import mybir
from concourse.bass_utils import run_bass_kernel_spmd

P = 128
S = 2048
E = 2048
D = 64
H = 32          # total q heads
KVH = 8         # total kv heads
HQ = 8          # q heads per core
NT = E // P     # contraction tiles
NQ = S // P     # q blocks (and k blocks)
SCALING = D ** -0.5
FP32 = mybir.dt.float32
FP32R = mybir.dt.float32r
BF16 = mybir.dt.bfloat16
AxX = mybir.AxisListType.X
Max = mybir.AluOpType.max
Exp = mybir.ActivationFunctionType.Exp

_PROGRAM_CACHE = {}


def _build_program(causal: bool, use_rowmax: bool, use_mask: bool):
    nc = bacc.Bacc()

    xt = nc.declare_dram_parameter("xt", [E, S], FP32R, isOutput=False)
    wq = nc.declare_dram_parameter("wq", [E, 512], FP32R, isOutput=False)
    wk = nc.declare_dram_parameter("wk", [E, 128], FP32R, isOutput=False)
    wv = nc.declare_dram_parameter("wv", [E, 128], FP32R, isOutput=False)
    wo = nc.declare_dram_parameter("wo", [512, E], FP32R, isOutput=False)
    cos2 = nc.declare_dram_parameter("cos2", [P, S], FP32, isOutput=False)
    sin2 = nc.declare_dram_parameter("sin2", [P, S], FP32, isOutput=False)
    rotT = nc.declare_dram_parameter("rotT", [P, P], FP32R, isOutput=False)
    ident = nc.declare_dram_parameter("ident", [P, P], FP32, isOutput=False)
    identb = nc.declare_dram_parameter("identb", [P, P], BF16, isOutput=False)
    tri = nc.declare_dram_parameter("tri", [P, P], FP32, isOutput=False)
    mask = None
    if use_mask:
        mask = nc.declare_dram_parameter("mask", [S, S], FP32, isOutput=False)
    attn_w = nc.declare_dram_parameter("attn_w", [HQ, S, S], FP32, isOutput=True)
    o_part = nc.declare_dram_parameter("o_part", [S, E], FP32, isOutput=True)

    with tile.TileContext(nc) as tc:
        with tc.tile_pool(name="consts", bufs=1) as consts, \
             tc.tile_pool(name="persist", bufs=1) as persist:
            ident_t = consts.tile([P, P], FP32)
            identb_t = consts.tile([P, P], BF16)
            tri_t = consts.tile([P, P], FP32)
            rotT_t = consts.tile([P, P], FP32R)
            bias0_t = consts.tile([P, 1], FP32)
            nc.sync.dma_start(out=ident_t, in_=ident[:, :])
            nc.sync.dma_start(out=identb_t, in_=identb[:, :])
            nc.sync.dma_start(out=tri_t, in_=tri[:, :])
            nc.sync.dma_start(out=rotT_t, in_=rotT[:, :])
            nc.vector.memset(bias0_t, 0.0)

            qkvT = persist.tile([P, 5, S], FP32R)      # m 0-3: q head pairs, 4: k
            v_bf = persist.tile([P, NQ, P], BF16)      # [S_k part, ki, 2kv*64]

            # ================= phase 1: qkv projection + rope =================
            with tc.tile_pool(name="ph1", bufs=2) as ph1, \
                 tc.tile_pool(name="ph1c", bufs=1) as ph1c, \
                 tc.tile_pool(name="ropet", bufs=3) as ropet, \
                 tc.tile_pool(name="qkv_ps", bufs=2, space="PSUM") as qkv_ps, \
                 tc.tile_pool(name="rot_ps", bufs=2, space="PSUM") as rot_ps:
                w_all = ph1c.tile([P, NT, 768], FP32R)
                wq_r = wq[:, :].rearrange("(t p) m -> t p m", p=P)
                wk_r = wk[:, :].rearrange("(t p) m -> t p m", p=P)
                wv_r = wv[:, :].rearrange("(t p) m -> t p m", p=P)
                for t in range(NT):
                    nc.sync.dma_start(out=w_all[:, t, 0:512], in_=wq_r[t])
                    nc.sync.dma_start(out=w_all[:, t, 512:640], in_=wk_r[t])
                    nc.sync.dma_start(out=w_all[:, t, 640:768], in_=wv_r[t])
                cos2_t = ph1c.tile([P, S], FP32)
                sin2_t = ph1c.tile([P, S], FP32)
                nc.sync.dma_start(out=cos2_t, in_=cos2[:, :])
                nc.sync.dma_start(out=sin2_t, in_=sin2[:, :])
                vT_stage = ph1c.tile([P, S], FP32)

                xt_r = xt[:, :].rearrange("(t p) s -> t p s", p=P)
                for sc in range(4):
                    sl = slice(sc * 512, (sc + 1) * 512)
                    xt_sl = ph1.tile([P, NT, 512], FP32R)
                    for t in range(NT):
                        nc.sync.dma_start(out=xt_sl[:, t, :], in_=xt_r[t, :, sl])
                    for m in range(6):
                        psq = qkv_ps.tile([P, 512], FP32)
                        for t in range(NT):
                            nc.tensor.matmul(psq, w_all[:, t, m * 128:(m + 1) * 128],
                                             xt_sl[:, t, :], start=(t == 0), stop=(t == NT - 1))
                        if m == 5:
                            nc.scalar.copy(vT_stage[:, sl], psq)
                        else:
                            nc.scalar.copy(qkvT[:, m, sl], psq)
                    for m in range(5):
                        rps = rot_ps.tile([P, 512], FP32)
                        nc.tensor.matmul(rps, rotT_t, qkvT[:, m, sl], start=True, stop=True)
                        t1 = ropet.tile([P, 512], FP32, tag="t1")
                        t2 = ropet.tile([P, 512], FP32, tag="t2")
                        nc.vector.tensor_mul(t1, rps, sin2_t[:, sl])
                        nc.vector.tensor_mul(t2, qkvT[:, m, sl], cos2_t[:, sl])
                        nc.vector.tensor_add(qkvT[:, m, sl], t1, t2)

                with tc.tile_pool(name="vt_ps", bufs=2, space="PSUM") as vt_ps:
                    for ki in range(NQ):
                        vps = vt_ps.tile([P, P], FP32)
                        nc.tensor.transpose(vps, vT_stage[:, ki * P:(ki + 1) * P], ident_t)
                        nc.scalar.copy(v_bf[:, ki, :], vps)

            # ================= phase 2+3 shared: attn_out^T =================
            at_pool_cm = tc.tile_pool(name="at_pool", bufs=1)
            at_pool = at_pool_cm.__enter__()
            AT = at_pool.tile([P, 4, S], FP32R)        # attn_out^T (normalized)

            # ================= phase 2: attention =================
            from contextlib import ExitStack
            _mask_ctx = ExitStack()
            mask_pool = _mask_ctx.enter_context(tc.tile_pool(name="mask_sb", bufs=3)) if use_mask else None
            with tc.tile_pool(name="pu", bufs=3) as pu_pool, \
                 tc.tile_pool(name="pn", bufs=3) as pn_pool, \
                 tc.tile_pool(name="pb", bufs=5) as pb_pool, \
                 tc.tile_pool(name="ptb", bufs=2) as ptb_pool, \
                 tc.tile_pool(name="rsmall", bufs=8) as r_pool, \
                 tc.tile_pool(name="sc_ps", bufs=2, space="PSUM") as sc_ps, \
                 tc.tile_pool(name="tp_ps", bufs=2, space="PSUM") as tp_ps, \
                 tc.tile_pool(name="av_ps", bufs=2, space="PSUM") as av_ps:
                for h in range(HQ):
                    half = h // 4
                    m = h % 4
                    pbase = 64 * half
                    qT_h = qkvT[pbase:pbase + 64, m, :]
                    kT_h = qkvT[pbase:pbase + 64, 4, :]
                    for g in range(4):
                        Pb_tiles = []
                        for qi in range(4 * g, 4 * g + 4):
                            ext = (qi + 1) * P if causal else S
                            npieces = (ext + 1023) // 1024
                            P_u = pu_pool.tile([P, S], FP32)
                            racc = r_pool.tile([P, 2], FP32, tag="racc")
                            rmx = r_pool.tile([P, 2], FP32, tag="rmx") if use_rowmax else None
                            pieces = []
                            for pc in range(npieces):
                                pl = 1024 * pc
                                pr = min(ext, pl + 1024)
                                w = pr - pl
                                ps_s = sc_ps.tile([P, 1024], FP32)
                                for ch in range(pl, pr, 512):
                                    cw = min(512, pr - ch)
                                    nc.tensor.matmul(ps_s[:, ch - pl:ch - pl + cw],
                                                     qT_h[:, qi * P:(qi + 1) * P],
                                                     kT_h[:, ch:ch + cw],
                                                     start=True, stop=True)
                                if use_mask:
                                    mt = mask_pool.tile([P, 1024], FP32)
                                    nc.sync.dma_start(out=mt[:, 0:w],
                                                      in_=mask[qi * P:(qi + 1) * P, pl:pr])
                                    nc.vector.tensor_add(ps_s[:, 0:w], ps_s[:, 0:w], mt[:, 0:w])
                                if causal and pl <= qi * P < pr:
                                    doff = qi * P - pl
                                    nc.vector.tensor_add(ps_s[:, doff:doff + P],
                                                         ps_s[:, doff:doff + P], tri_t)
                                if use_rowmax:
                                    nc.vector.tensor_reduce(rmx[:, pc:pc + 1], ps_s[:, 0:w],
                                                            axis=AxX, op=Max)
                                    pieces.append((ps_s, pl, pr, w))
                                else:
                                    nc.scalar.activation(P_u[:, pl:pr], ps_s[:, 0:w], Exp,
                                                         bias=bias0_t, scale=SCALING,
                                                         accum_out=racc[:, pc:pc + 1])
                            if use_rowmax:
                                mneg = r_pool.tile([P, 1], FP32, tag="mneg")
                                if npieces == 2:
                                    mcmb = r_pool.tile([P, 1], FP32, tag="mcmb")
                                    nc.vector.tensor_reduce(mcmb, rmx[:, 0:2], axis=AxX, op=Max)
                                    nc.vector.tensor_scalar_mul(mneg, mcmb, -SCALING)
                                else:
                                    nc.vector.tensor_scalar_mul(mneg, rmx[:, 0:1], -SCALING)
                                for (ps_s, pl, pr, w) in pieces:
                                    pc = pl // 1024
                                    nc.scalar.activation(P_u[:, pl:pr], ps_s[:, 0:w], Exp,
                                                         bias=mneg, scale=SCALING,
                                                         accum_out=racc[:, pc:pc + 1])
                            recip = r_pool.tile([P, 1], FP32, tag="recip")
                            if npieces == 2:
                                rs = r_pool.tile([P, 1], FP32, tag="rs")
                                nc.vector.tensor_reduce(rs, racc[:, 0:2], axis=AxX,
                                                        op=mybir.AluOpType.add)
                                nc.vector.reciprocal(recip, rs)
                            else:
                                nc.vector.reciprocal(recip, racc[:, 0:1])
                            P_n = pn_pool.tile([P, S], FP32)
                            nc.vector.tensor_scalar_mul(P_n[:, 0:ext], P_u[:, 0:ext], recip)
                            nc.sync.dma_start(out=attn_w[h, qi * P:(qi + 1) * P, 0:ext],
                                              in_=P_n[:, 0:ext])
                            Pb = pb_pool.tile([P, S], BF16)
                            nc.gpsimd.tensor_copy(Pb[:, 0:ext], P_n[:, 0:ext])
                            Pb_tiles.append(Pb)

                        # ---- transpose P (bf16) and A = P @ V ----
                        ki_hi = 4 * g + 4 if causal else NQ
                        PTb_t = ptb_pool.tile([P, NQ, 512], BF16)
                        for kb in range(0, ki_hi, 2):
                            pst = tp_ps.tile([P, 1024], BF16)
                            for ko in range(2):
                                ki = kb + ko
                                if ki >= ki_hi:
                                    continue
                                for j in range(4):
                                    qi = 4 * g + j
                                    if causal and qi < ki:
                                        continue
                                    nc.tensor.transpose(pst[:, 512 * ko + 128 * j:512 * ko + 128 * (j + 1)],
                                                        Pb_tiles[j][:, ki * P:(ki + 1) * P],
                                                        identb_t)
                            for ko in range(2):
                                ki = kb + ko
                                if ki >= ki_hi:
                                    continue
                                inv = max(0, ki - 4 * g) if causal else 0
                                if inv > 0:
                                    nc.gpsimd.memset(PTb_t[:, ki, 0:128 * inv], 0.0)
                                nc.vector.tensor_copy(PTb_t[:, ki, 128 * inv:512],
                                                      pst[:, 512 * ko + 128 * inv:512 * (ko + 1)])
                        ps_av = av_ps.tile([64, 512], FP32)
                        for ki in range(ki_hi):
                            nc.tensor.matmul(ps_av, v_bf[:, ki, pbase:pbase + 64],
                                             PTb_t[:, ki, :],
                                             start=(ki == 0), stop=(ki == ki_hi - 1))
                        nc.scalar.copy(AT[64 * (h % 2):64 * (h % 2) + 64, h // 2,
                                          g * 512:(g + 1) * 512], ps_av)

            # ================= phase 3: o_proj =================
            with tc.tile_pool(name="ph3", bufs=1) as ph3, \
                 tc.tile_pool(name="osb", bufs=2) as osb_pool, \
                 tc.tile_pool(name="o_ps", bufs=2, space="PSUM") as o_ps:
                wo_t = ph3.tile([P, 4, S], FP32R)
                wo_r = wo[:, :].rearrange("(t p) e -> t p e", p=P)
                for t in range(4):
                    nc.sync.dma_start(out=wo_t[:, t, :], in_=wo_r[t])
                for st in range(NQ):
                    o_sb = osb_pool.tile([P, S], FP32)
                    for nch in range(4):
                        pso = o_ps.tile([P, 512], FP32)
                        for kt in range(4):
                            nc.tensor.matmul(pso, AT[:, kt, st * P:(st + 1) * P],
                                             wo_t[:, kt, nch * 512:(nch + 1) * 512],
                                             start=(kt == 0), stop=(kt == 3))
                        nc.scalar.copy(o_sb[:, nch * 512:(nch + 1) * 512], pso)
                    nc.sync.dma_start(out=o_part[st * P:(st + 1) * P, :], in_=o_sb)
            if use_mask:
                _mask_ctx.close()
            at_pool_cm.__exit__(None, None, None)

    nc.compile()
    return nc


def _get_program(key):
    if key not in _PROGRAM_CACHE:
        _PROGRAM_CACHE[key] = _build_program(*key)
    return _PROGRAM_CACHE[key]


def _host_consts():
    ident = np.eye(P, dtype=np.float32)
    identb = np.eye(P, dtype=ml_dtypes.bfloat16)
    # R2^T where R2 = blockdiag(B, B), B = [[0, -I32], [I32, 0]] (64x64)
    B = np.zeros((64, 64), np.float32)
    B[np.arange(32), np.arange(32) + 32] = -1.0
    B[np.arange(32) + 32, np.arange(32)] = 1.0
    R2 = np.zeros((P, P), np.float32)
    R2[0:64, 0:64] = B
    R2[64:128, 64:128] = B
    rotT = np.ascontiguousarray(R2.T)
    # additive tri mask for the diagonal block (raw-score units)
    tri_m = np.where(np.tril(np.ones((P, P), bool)), 0.0, -1e9).astype(np.float32)
    return ident, identb, rotT, tri_m


def kernel(hidden_states, attention_mask, cos, sin, wq, wk, wv, wo):
    hidden_states = np.asarray(hidden_states, dtype=np.float32)
    attention_mask = np.asarray(attention_mask, dtype=np.float32)
    cos = np.asarray(cos, dtype=np.float32)
    sin = np.asarray(sin, dtype=np.float32)
    wq = np.asarray(wq, dtype=np.float32)
    wk = np.asarray(wk, dtype=np.float32)
    wv = np.asarray(wv, dtype=np.float32)
    wo = np.asarray(wo, dtype=np.float32)
    B = hidden_states.shape[0]
    assert hidden_states.shape == (B, S, E)

    # ---- classify the mask ----
    m0 = attention_mask[0, 0]
    causal_ref = np.where(np.tril(np.ones((S, S), bool)), 0.0, -1e9).astype(np.float32)
    is_zero = not attention_mask.any()
    is_causal = (not is_zero) and all(
        np.array_equal(attention_mask[b, 0], causal_ref) for b in range(B))
    use_mask = not (is_zero or is_causal)

    # ---- decide whether per-row max subtraction is needed ----
    sd_h = float(hidden_states.std())
    sd_q = np.sqrt(E) * float(wq.std()) * sd_h
    sd_k = np.sqrt(E) * float(wk.std()) * sd_h
    rope_f = np.sqrt(float((cos ** 2).mean()) + float((sin ** 2).mean()))
    sigma_s = np.sqrt(D) * (sd_q * rope_f) * (sd_k * rope_f) * SCALING
    use_rowmax = bool(use_mask or sigma_s > 6.0)

    key = (bool(is_causal), use_rowmax, use_mask)
    nc = _get_program(key)

    ident, identb, rotT, tri_m = _host_consts()
    groups = H // KVH  # 4

    in_maps = []
    for c in range(8):
        b = c // 4
        g = c % 4
        heads = [8 * g + hh for hh in range(HQ)]
        # m-tile order pairs local head m with m+4
        order = []
        for mm in range(4):
            order += [heads[mm], heads[mm + 4]]
        wq_c = np.concatenate([wq[:, hh * D:(hh + 1) * D] for hh in order], axis=1)
        kvs = [2 * g, 2 * g + 1]
        wk_c = np.concatenate([wk[:, kv * D:(kv + 1) * D] for kv in kvs], axis=1)
        wv_c = np.concatenate([wv[:, kv * D:(kv + 1) * D] for kv in kvs], axis=1)
        wo_c = wo[8 * g * D:(8 * g + 8) * D, :]
        cosT = cos[b].T  # [D, S]
        sinT = sin[b].T
        im = {
            "xt": np.ascontiguousarray(hidden_states[b].T),
            "wq": np.ascontiguousarray(wq_c),
            "wk": np.ascontiguousarray(wk_c),
            "wv": np.ascontiguousarray(wv_c),
            "wo": np.ascontiguousarray(wo_c),
            "cos2": np.ascontiguousarray(np.concatenate([cosT, cosT], axis=0)),
            "sin2": np.ascontiguousarray(np.concatenate([sinT, sinT], axis=0)),
            "rotT": rotT,
            "ident": ident,
            "identb": identb,
            "tri": tri_m,
        }
        if use_mask:
            im["mask"] = np.ascontiguousarray(attention_mask[b, 0] / SCALING)
        in_maps.append(im)

    res = run_bass_kernel_spmd(nc, in_maps, core_ids=list(range(8)))

    attn_output = np.zeros((B, S, E), np.float32)
    attn_weights = np.empty((B, H, S, S), np.float32)
    for c in range(8):
        b = c // 4
        g = c % 4
        attn_output[b] += res.results[c]["o_part"]
        for hh in range(HQ):
            attn_weights[b, 8 * g + hh] = res.results[c]["attn_w"][hh]
    return attn_output, attn_weights


# revision 10
# speedup vs baseline: 1.0834x; 1.0834x over previous
"""Llama GQA attention (B=2, S=2048, E=2048, H=32, KVH=8, D=64) on 8 TRN2 cores.

Sharding: data-parallel over batch x tensor-parallel over heads. Core c
handles batch c//4 and q-heads 8*(c%4)..8*(c%4)+7 (kv heads 2*(c%4)+{0,1}).
o_proj is row-parallel; the 4 partial outputs per batch are summed on host.

kernel(**inputs) takes the full unsharded inputs and returns
(attn_output [B,S,E] f32, attn_weights [B,H,S,S] f32) like the reference.
"""
import sys
sys.path.insert(0, "/opt/trn_rl_repo")

import numpy as np
import ml_dtypes

import concourse.bass as bass
import concourse.bacc as bacc
import concourse.tile as tile
from concourse import mybir
from concourse.bass_utils import run_bass_kernel_spmd

P = 128
S = 2048
E = 2048
D = 64
H = 32          # total q heads
KVH = 8         # total kv heads
HQ = 8          # q heads per core
NT = E // P     # contraction tiles
NQ = S // P     # q blocks (and k blocks)
SCALING = D ** -0.5
FP32 = mybir.dt.float32
FP32R = mybir.dt.float32r
BF16 = mybir.dt.bfloat16
AxX = mybir.AxisListType.X
Max = mybir.AluOpType.max
Exp = mybir.ActivationFunctionType.Exp

_PROGRAM_CACHE = {}


def _build_program(causal: bool, use_rowmax: bool, use_mask: bool):
    nc = bacc.Bacc()

    xt = nc.declare_dram_parameter("xt", [E, S], FP32R, isOutput=False)
    wq = nc.declare_dram_parameter("wq", [E, 512], FP32R, isOutput=False)
    wk = nc.declare_dram_parameter("wk", [E, 128], FP32R, isOutput=False)
    wv = nc.declare_dram_parameter("wv", [E, 128], FP32R, isOutput=False)
    wo = nc.declare_dram_parameter("wo", [512, E], FP32R, isOutput=False)
    cos2 = nc.declare_dram_parameter("cos2", [P, S], FP32, isOutput=False)
    sin2 = nc.declare_dram_parameter("sin2", [P, S], FP32, isOutput=False)
    rotT = nc.declare_dram_parameter("rotT", [P, P], FP32R, isOutput=False)
    ident = nc.declare_dram_parameter("ident", [P, P], FP32, isOutput=False)
    identr = nc.declare_dram_parameter("identr", [P, P], FP32R, isOutput=False)
    identb = nc.declare_dram_parameter("identb", [P, P], BF16, isOutput=False)
    tri = nc.declare_dram_parameter("tri", [P, P], FP32R, isOutput=False)
    mask = None
    if use_mask:
        mask = nc.declare_dram_parameter("mask", [S, S], FP32, isOutput=False)
    attn_w = nc.declare_dram_parameter("attn_w", [HQ, S, S], FP32, isOutput=True)
    o_part = nc.declare_dram_parameter("o_part", [S, E], FP32, isOutput=True)

    with tile.TileContext(nc) as tc:
        with tc.tile_pool(name="consts", bufs=1) as consts, \
             tc.tile_pool(name="persist", bufs=1) as persist:
            ident_t = consts.tile([P, P], FP32)
            identb_t = consts.tile([P, P], BF16)
            tri_t = consts.tile([P, P], FP32R)
            identr_t = consts.tile([P, P], FP32R)
            rotT_t = consts.tile([P, P], FP32R)
            bias0_t = consts.tile([P, 1], FP32)
            nc.sync.dma_start(out=ident_t, in_=ident[:, :])
            nc.sync.dma_start(out=identb_t, in_=identb[:, :])
            nc.sync.dma_start(out=tri_t, in_=tri[:, :])
            nc.sync.dma_start(out=identr_t, in_=identr[:, :])
            nc.sync.dma_start(out=rotT_t, in_=rotT[:, :])
            nc.vector.memset(bias0_t, 0.0)

            qkvT = persist.tile([P, 5, S], FP32R)      # m 0-3: q head pairs, 4: k
            v_bf = persist.tile([P, NQ, P], BF16)      # [S_k part, ki, 2kv*64]

            # ================= phase 1: qkv projection + rope =================
            with tc.tile_pool(name="ph1", bufs=2) as ph1, \
                 tc.tile_pool(name="ph1c", bufs=1) as ph1c, \
                 tc.tile_pool(name="ropet", bufs=3) as ropet, \
                 tc.tile_pool(name="qkv_ps", bufs=2, space="PSUM") as qkv_ps, \
                 tc.tile_pool(name="rot_ps", bufs=2, space="PSUM") as rot_ps:
                w_all = ph1c.tile([P, NT, 768], FP32R)
                wq_r = wq[:, :].rearrange("(t p) m -> t p m", p=P)
                wk_r = wk[:, :].rearrange("(t p) m -> t p m", p=P)
                wv_r = wv[:, :].rearrange("(t p) m -> t p m", p=P)
                for t in range(NT):
                    nc.sync.dma_start(out=w_all[:, t, 0:512], in_=wq_r[t])
                    nc.sync.dma_start(out=w_all[:, t, 512:640], in_=wk_r[t])
                    nc.sync.dma_start(out=w_all[:, t, 640:768], in_=wv_r[t])
                cos2_t = ph1c.tile([P, S], FP32)
                sin2_t = ph1c.tile([P, S], FP32)
                nc.sync.dma_start(out=cos2_t, in_=cos2[:, :])
                nc.sync.dma_start(out=sin2_t, in_=sin2[:, :])
                vT_stage = ph1c.tile([P, S], FP32)

                xt_r = xt[:, :].rearrange("(t p) s -> t p s", p=P)
                for sc in range(4):
                    sl = slice(sc * 512, (sc + 1) * 512)
                    xt_sl = ph1.tile([P, NT, 512], FP32R)
                    for t in range(NT):
                        nc.sync.dma_start(out=xt_sl[:, t, :], in_=xt_r[t, :, sl])
                    for m in range(6):
                        psq = qkv_ps.tile([P, 512], FP32)
                        for t in range(NT):
                            nc.tensor.matmul(psq, w_all[:, t, m * 128:(m + 1) * 128],
                                             xt_sl[:, t, :], start=(t == 0), stop=(t == NT - 1))
                        if m == 5:
                            nc.scalar.copy(vT_stage[:, sl], psq)
                        else:
                            nc.scalar.copy(qkvT[:, m, sl], psq)
                    for m in range(5):
                        rps = rot_ps.tile([P, 512], FP32)
                        nc.tensor.matmul(rps, rotT_t, qkvT[:, m, sl], start=True, stop=True)
                        t1 = ropet.tile([P, 512], FP32, tag="t1")
                        t2 = ropet.tile([P, 512], FP32, tag="t2")
                        nc.vector.tensor_mul(t1, rps, sin2_t[:, sl])
                        nc.vector.tensor_mul(t2, qkvT[:, m, sl], cos2_t[:, sl])
                        nc.vector.tensor_add(qkvT[:, m, sl], t1, t2)

                with tc.tile_pool(name="vt_ps", bufs=2, space="PSUM") as vt_ps:
                    for ki in range(NQ):
                        vps = vt_ps.tile([P, P], FP32)
                        nc.tensor.transpose(vps, vT_stage[:, ki * P:(ki + 1) * P], ident_t)
                        nc.scalar.copy(v_bf[:, ki, :], vps)

            # ================= phase 2+3 shared: attn_out^T =================
            at_pool_cm = tc.tile_pool(name="at_pool", bufs=1)
            at_pool = at_pool_cm.__enter__()
            AT = at_pool.tile([P, 4, S], FP32R)        # attn_out^T (normalized)

            # ================= phase 2: attention =================
            pair = not (use_rowmax or use_mask)
            from contextlib import ExitStack
            _mask_ctx = ExitStack()
            mask_pool = _mask_ctx.enter_context(tc.tile_pool(name="mask_sb", bufs=3)) if use_mask else None
            with tc.tile_pool(name="pu", bufs=2) as pu_pool, \
                 tc.tile_pool(name="pn", bufs=1) as pn_pool, \
                 tc.tile_pool(name="pb", bufs=4) as pb_pool, \
                 tc.tile_pool(name="ptb", bufs=1) as ptb_pool, \
                 tc.tile_pool(name="rsmall", bufs=8) as r_pool, \
                 tc.tile_pool(name="sc_ps", bufs=(1 if pair else 2), space="PSUM") as sc_ps, \
                 tc.tile_pool(name="tp_ps", bufs=1, space="PSUM") as tp_ps, \
                 tc.tile_pool(name="av_ps", bufs=1, space="PSUM") as av_ps:
                def emit_scores_exp(h, qi, state):
                    """Scores + exp + normalize + bf16 cast for one (h, qi)."""
                    half = h // 4
                    hs = half if pair else 0
                    m = h % 4
                    pbase = 64 * half
                    qT_h = qkvT[pbase:pbase + 64, m, :]
                    kT_h = qkvT[pbase:pbase + 64, 4, :]
                    ext = (qi + 1) * P if causal else S
                    npieces = (ext + 1023) // 1024
                    P_u = pu_pool.tile([P, S], FP32, tag=f"pu{hs}")
                    racc = r_pool.tile([P, 2], FP32, tag=f"racc{hs}")
                    rmx = r_pool.tile([P, 2], FP32, tag=f"rmx{hs}") if use_rowmax else None
                    pieces = []
                    for pc in range(npieces):
                        pl = 1024 * pc
                        pr = min(ext, pl + 1024)
                        w = pr - pl
                        ps_s = sc_ps.tile([P, 1024], FP32, tag=f"ps_s{hs}")
                        for ch in range(pl, pr, 512):
                            cw = min(512, pr - ch)
                            has_diag = causal and ch <= qi * P < ch + cw
                            nc.tensor.matmul(ps_s[:, ch - pl:ch - pl + cw],
                                             qT_h[:, qi * P:(qi + 1) * P],
                                             kT_h[:, ch:ch + cw],
                                             start=True, stop=not has_diag)
                            if has_diag:
                                doff = qi * P - ch
                                nc.tensor.matmul(ps_s[:, ch - pl + doff:ch - pl + doff + P],
                                                 identr_t, tri_t,
                                                 start=False, stop=True,
                                                 skip_group_check=True)
                        if use_mask:
                            mt = mask_pool.tile([P, 1024], FP32)
                            nc.sync.dma_start(out=mt[:, 0:w],
                                              in_=mask[qi * P:(qi + 1) * P, pl:pr])
                            nc.vector.tensor_add(ps_s[:, 0:w], ps_s[:, 0:w], mt[:, 0:w])
                        if use_rowmax:
                            nc.vector.tensor_reduce(rmx[:, pc:pc + 1], ps_s[:, 0:w],
                                                    axis=AxX, op=Max)
                            pieces.append((ps_s, pl, pr, w))
                        else:
                            nc.scalar.activation(P_u[:, pl:pr], ps_s[:, 0:w], Exp,
                                                 bias=bias0_t, scale=SCALING,
                                                 accum_out=racc[:, pc:pc + 1])
                    if use_rowmax:
                        mneg = r_pool.tile([P, 1], FP32, tag="mneg")
                        if npieces == 2:
                            mcmb = r_pool.tile([P, 1], FP32, tag="mcmb")
                            nc.vector.tensor_reduce(mcmb, rmx[:, 0:2], axis=AxX, op=Max)
                            nc.vector.tensor_scalar_mul(mneg, mcmb, -SCALING)
                        else:
                            nc.vector.tensor_scalar_mul(mneg, rmx[:, 0:1], -SCALING)
                        for (ps_s, pl, pr, w) in pieces:
                            pc = pl // 1024
                            nc.scalar.activation(P_u[:, pl:pr], ps_s[:, 0:w], Exp,
                                                 bias=mneg, scale=SCALING,
                                                 accum_out=racc[:, pc:pc + 1])
                    recip = r_pool.tile([P, 1], FP32, tag=f"recip{hs}")
                    if npieces == 2:
                        rs = r_pool.tile([P, 1], FP32, tag=f"rs{hs}")
                        nc.vector.tensor_reduce(rs, racc[:, 0:2], axis=AxX,
                                                op=mybir.AluOpType.add)
                        nc.vector.reciprocal(recip, rs)
                    else:
                        nc.vector.reciprocal(recip, racc[:, 0:1])
                    P_n = pn_pool.tile([P, S], FP32, tag=f"pn{hs}")
                    # normalize: alternate between ACT and DVE to balance load
                    if qi % 2 == 0:
                        nc.scalar.mul(P_n[:, 0:ext], P_u[:, 0:ext], recip)
                    else:
                        nc.vector.tensor_scalar_mul(P_n[:, 0:ext], P_u[:, 0:ext], recip)
                    nc.sync.dma_start(out=attn_w[h, qi * P:(qi + 1) * P, 0:ext],
                                      in_=P_n[:, 0:ext])
                    Pb = pb_pool.tile([P, S], BF16, tag=f"pb{hs}")
                    nc.vector.tensor_copy(Pb[:, 0:ext], P_n[:, 0:ext])
                    state.append(Pb)

                def emit_trans_av(h, g, Pb_tiles):
                    half = h // 4
                    hs = half if pair else 0
                    pbase = 64 * half
                    ki_hi = 4 * g + 4 if causal else NQ
                    PTb_t = ptb_pool.tile([P, NQ, 512], BF16, tag=f"ptb{hs}")
                    for kb in range(0, ki_hi, 2):
                        pst = tp_ps.tile([P, 1024], BF16, tag=f"pst{hs}")
                        for ko in range(2):
                            ki = kb + ko
                            if ki >= ki_hi:
                                continue
                            for j in range(4):
                                qi = 4 * g + j
                                if causal and qi < ki:
                                    continue
                                nc.tensor.transpose(pst[:, 512 * ko + 128 * j:512 * ko + 128 * (j + 1)],
                                                    Pb_tiles[j][:, ki * P:(ki + 1) * P],
                                                    identb_t)
                        for ko in range(2):
                            ki = kb + ko
                            if ki >= ki_hi:
                                continue
                            inv = max(0, ki - 4 * g) if causal else 0
                            if inv > 0:
                                nc.gpsimd.memset(PTb_t[:, ki, 0:128 * inv], 0.0)
                            nc.vector.tensor_copy(PTb_t[:, ki, 128 * inv:512],
                                                  pst[:, 512 * ko + 128 * inv:512 * (ko + 1)])
                    ps_av = av_ps.tile([64, 512], FP32, tag=f"av{hs}")
                    for ki in range(ki_hi):
                        nc.tensor.matmul(ps_av, v_bf[:, ki, pbase:pbase + 64],
                                         PTb_t[:, ki, :],
                                         start=(ki == 0), stop=(ki == ki_hi - 1))
                    nc.scalar.copy(AT[64 * (h % 2):64 * (h % 2) + 64, h // 2,
                                      g * 512:(g + 1) * 512], ps_av)

                # interleave the (hp, hp+4) head pair: their K=64 score matmuls
                # sit in disjoint PE row-strips and run concurrently
                if pair:
                    for hp in range(4):
                        for g in range(4):
                            states = ([], [])
                            for qi in range(4 * g, 4 * g + 4):
                                emit_scores_exp(hp, qi, states[0])
                                emit_scores_exp(hp + 4, qi, states[1])
                            emit_trans_av(hp, g, states[0])
                            emit_trans_av(hp + 4, g, states[1])
                else:
                    for h in range(HQ):
                        for g in range(4):
                            state = []
                            for qi in range(4 * g, 4 * g + 4):
                                emit_scores_exp(h, qi, state)
                            emit_trans_av(h, g, state)

            # ================= phase 3: o_proj =================
            with tc.tile_pool(name="ph3", bufs=1) as ph3, \
                 tc.tile_pool(name="osb", bufs=2) as osb_pool, \
                 tc.tile_pool(name="o_ps", bufs=2, space="PSUM") as o_ps:
                wo_t = ph3.tile([P, 4, S], FP32R)
                wo_r = wo[:, :].rearrange("(t p) e -> t p e", p=P)
                for t in range(4):
                    nc.sync.dma_start(out=wo_t[:, t, :], in_=wo_r[t])
                for st in range(NQ):
                    o_sb = osb_pool.tile([P, S], FP32)
                    for nch in range(4):
                        pso = o_ps.tile([P, 512], FP32)
                        for kt in range(4):
                            nc.tensor.matmul(pso, AT[:, kt, st * P:(st + 1) * P],
                                             wo_t[:, kt, nch * 512:(nch + 1) * 512],
                                             start=(kt == 0), stop=(kt == 3))
                        nc.vector.tensor_copy(o_sb[:, nch * 512:(nch + 1) * 512], pso)
                    nc.sync.dma_start(out=o_part[st * P:(st + 1) * P, :], in_=o_sb)
            if use_mask:
                _mask_ctx.close()
            at_pool_cm.__exit__(None, None, None)

    nc.compile()
    return nc


def _get_program(key):
    if key not in _PROGRAM_CACHE:
        _PROGRAM_CACHE[key] = _build_program(*key)
    return _PROGRAM_CACHE[key]


def _host_consts():
    ident = np.eye(P, dtype=np.float32)
    identb = np.eye(P, dtype=ml_dtypes.bfloat16)
    # R2^T where R2 = blockdiag(B, B), B = [[0, -I32], [I32, 0]] (64x64)
    B = np.zeros((64, 64), np.float32)
    B[np.arange(32), np.arange(32) + 32] = -1.0
    B[np.arange(32) + 32, np.arange(32)] = 1.0
    R2 = np.zeros((P, P), np.float32)
    R2[0:64, 0:64] = B
    R2[64:128, 64:128] = B
    rotT = np.ascontiguousarray(R2.T)
    # additive tri mask for the diagonal block (raw-score units)
    tri_m = np.where(np.tril(np.ones((P, P), bool)), 0.0, -1e9).astype(np.float32)
    return ident, identb, rotT, tri_m


def kernel(hidden_states, attention_mask, cos, sin, wq, wk, wv, wo):
    hidden_states = np.asarray(hidden_states, dtype=np.float32)
    attention_mask = np.asarray(attention_mask, dtype=np.float32)
    cos = np.asarray(cos, dtype=np.float32)
    sin = np.asarray(sin, dtype=np.float32)
    wq = np.asarray(wq, dtype=np.float32)
    wk = np.asarray(wk, dtype=np.float32)
    wv = np.asarray(wv, dtype=np.float32)
    wo = np.asarray(wo, dtype=np.float32)
    B = hidden_states.shape[0]
    assert hidden_states.shape == (B, S, E)

    # ---- classify the mask ----
    m0 = attention_mask[0, 0]
    causal_ref = np.where(np.tril(np.ones((S, S), bool)), 0.0, -1e9).astype(np.float32)
    is_zero = not attention_mask.any()
    is_causal = (not is_zero) and all(
        np.array_equal(attention_mask[b, 0], causal_ref) for b in range(B))
    use_mask = not (is_zero or is_causal)

    # ---- decide whether per-row max subtraction is needed ----
    sd_h = float(hidden_states.std())
    sd_q = np.sqrt(E) * float(wq.std()) * sd_h
    sd_k = np.sqrt(E) * float(wk.std()) * sd_h
    rope_f = np.sqrt(float((cos ** 2).mean()) + float((sin ** 2).mean()))
    sigma_s = np.sqrt(D) * (sd_q * rope_f) * (sd_k * rope_f) * SCALING
    use_rowmax = bool(use_mask or sigma_s > 6.0)

    key = (bool(is_causal), use_rowmax, use_mask)
    nc = _get_program(key)

    ident, identb, rotT, tri_m = _host_consts()
    groups = H // KVH  # 4

    in_maps = []
    for c in range(8):
        b = c // 4
        g = c % 4
        heads = [8 * g + hh for hh in range(HQ)]
        # m-tile order pairs local head m with m+4
        order = []
        for mm in range(4):
            order += [heads[mm], heads[mm + 4]]
        wq_c = np.concatenate([wq[:, hh * D:(hh + 1) * D] for hh in order], axis=1)
        kvs = [2 * g, 2 * g + 1]
        wk_c = np.concatenate([wk[:, kv * D:(kv + 1) * D] for kv in kvs], axis=1)
        wv_c = np.concatenate([wv[:, kv * D:(kv + 1) * D] for kv in kvs], axis=1)
        wo_c = wo[8 * g * D:(8 * g + 8) * D, :]
        cosT = cos[b].T  # [D, S]
        sinT = sin[b].T
        im = {
            "identr": ident,
            "xt": np.ascontiguousarray(hidden_states[b].T),
            "wq": np.ascontiguousarray(wq_c),
            "wk": np.ascontiguousarray(wk_c),
            "wv": np.ascontiguousarray(wv_c),
            "wo": np.ascontiguousarray(wo_c),
            "cos2": np.ascontiguousarray(np.concatenate([cosT, cosT], axis=0)),
            "sin2": np.ascontiguousarray(np.concatenate([sinT, sinT], axis=0)),
            "rotT": rotT,
            "ident": ident,
            "identb": identb,
            "tri": tri_m,
        }
        if use_mask:
            im["mask"] = np.ascontiguousarray(attention_mask[b, 0] / SCALING)
        in_maps.append(im)

    res = run_bass_kernel_spmd(nc, in_maps, core_ids=list(range(8)))

    attn_output = np.zeros((B, S, E), np.float32)
    attn_weights = np.empty((B, H, S, S), np.float32)
    for c in range(8):
        b = c // 4
        g = c % 4
        attn_output[b] += res.results[c]["o_part"]
        for hh in range(HQ):
            attn_weights[b, 8 * g + hh] = res.results[c]["attn_w"][hh]
    return attn_output, attn_weights
